# revision 1
# baseline (speedup 1.0000x reference)
"""Trainium2 Bass kernel for nn_CapsuleLayer: 2x2 conv (128->1024ch) + dynamic routing.

Strategy (data-parallel over batch, 4 samples per core on 8 cores):
  - Conv as 4 accumulated fp32r matmuls per tile: K=in_channels(128) on partitions.
    The stationary operand must be a single contiguous free dim, so each tile
    covers a contiguous 127-wide window of x = two conv-output rows plus one
    fake position (index 63, the w=63 wrap); fake positions get their routing
    weights zeroed, so they never contribute.
  - Output u^T[n, oc] (n-chunk on partitions, oc=1024 free) in bf16, bias fused
    into the PSUM->SBUF cast copy.
  - PE-transpose u^T into u_A[oc, n] (the routing's two einsums contract over
    different axes; the PE only contracts the partition dim, so both layouts
    are needed). Row-sums for routing iteration 0 (uniform softmax) ride along
    on the transpose copies via ACT accum_out, with fake-column corrections
    subtracted afterwards.
  - Routing iterations as matmuls: the b-update uses u_A chunks as stationary
    weights, producing b^T[n, i] directly (softmax-friendly layout); softmax is
    ACT exp + DVE reduce/reciprocal; the o-einsum streams u^T with c^T
    stationary; block-diagonal extraction via constant mask + strided reduce.
  - The next sample's conv is emitted before the current sample's squash tail
    so the PE rolls straight into it.
"""
import os
import sys

sys.path.insert(0, "/opt/trn_rl_repo")

import numpy as np
import ml_dtypes

B, IN_C, H, W_SP = 32, 128, 64, 64
NUM_CAPS, D0, D1 = 32, 4, 8
OUT_C = NUM_CAPS * D0 * D1  # 1024
N_CORES = 8
SPC = B // N_CORES          # samples per core = 4
HO = WO = 63                # conv output spatial
NCH = 32                    # n-chunks per sample (2 output rows each; last = 1)
NV = 127                    # rows per chunk: 63 + fake + 63 (last: 63 + 64 junk)
XW = H * W_SP + 128         # x padded so the last window stays in bounds
GC = 16                     # chunks per softmax group
NG = NCH // GC              # groups per pass

# bench: repeat the whole body KREPS times on-device (timing only)
KREPS = int(os.environ.get("KREPS", "1"))

_BUILT = {}


def _build_nc():
    import concourse.bacc as bacc
    import concourse.mybir as mybir
    import concourse.tile as tile

    F32 = mybir.dt.float32
    F32R = mybir.dt.float32r
    BF16 = mybir.dt.bfloat16
    AF = mybir.ActivationFunctionType

    nc = bacc.Bacc("TRN2")

    xs = nc.dram_tensor("xs", [SPC, 128, XW], F32R, kind="ExternalInput")
    wt = nc.dram_tensor("wt", [128, 4 * OUT_C], F32R, kind="ExternalInput")
    bias_bc = nc.dram_tensor("bias_bc", [128, OUT_C], F32, kind="ExternalInput")
    # f32 consts: maskbd [128, 8*32] | mask2 [128, 32] | maskdiag [128, 1024]
    # (rows 0:32) | rmA [128, GC] | rmB [128, GC]
    cons_f = nc.dram_tensor("cons_f", [128, 256 + 32 + 1024 + 2 * GC], F32,
                            kind="ExternalInput")
    # bf16 consts: eye [128,128] | irep [128,128] (rows 0:32 used) | rhs5 [128, 32]
    cons_h = nc.dram_tensor("cons_h", [128, 288], BF16, kind="ExternalInput")
    out_d = nc.dram_tensor("out", [SPC, 32, 32], F32, kind="ExternalOutput")

    with nc.allow_low_precision("u is stored in bf16 by design"):
        with tile.TileContext(nc) as tc:
            _emit(nc, tc, mybir, F32, F32R, BF16, AF,
                  xs, wt, bias_bc, cons_f, cons_h, out_d)
    nc.compile()
    return nc


def _emit(nc, tc, mybir, F32, F32R, BF16, AF, xs, wt, bias_bc, cons_f, cons_h, out_d):
    from contextlib import ExitStack

    with ExitStack() as ctx:
        const = ctx.enter_context(tc.tile_pool(name="const", bufs=1))
        big = ctx.enter_context(tc.tile_pool(name="big", bufs=1))
        xpool = ctx.enter_context(tc.tile_pool(name="xp", bufs=2))
        work = ctx.enter_context(tc.tile_pool(name="work", bufs=2))
        ps = ctx.enter_context(tc.tile_pool(name="ps", bufs=1, space="PSUM"))

        wt_t = const.tile([128, 4 * OUT_C], F32R)
        bias_t = const.tile([128, OUT_C], F32)
        cf_t = const.tile([128, 256 + 32 + 1024 + 2 * GC], F32)
        ch_t = const.tile([128, 288], BF16)
        nc.gpsimd.dma_start(wt_t[:], wt[:, :])
        nc.gpsimd.dma_start(bias_t[:], bias_bc[:, :])
        nc.gpsimd.dma_start(cf_t[:], cons_f[:, :])
        nc.gpsimd.dma_start(ch_t[:], cons_h[:, :])
        maskbd = cf_t[:, 0:256]
        mask2 = cf_t[:, 256:288]
        maskdiag = cf_t[0:32, 288:1312]
        rmA = cf_t[:, 1312:1312 + GC]
        rmB = cf_t[:, 1312 + GC:1312 + 2 * GC]
        eye = ch_t[:, 0:128]
        irep = ch_t[0:32, 128:256]
        rhs5 = ch_t[:, 256:288]

        ut = big.tile([128, NCH, OUT_C], BF16)        # u^T: [n-part, chunk, oc]
        ua = big.tile([128, 8, NCH, 128], BF16)       # u_A: [oc-part, g, chunk, n]
        nc.vector.memset(ua[:], 0.0)
        out_sb = big.tile([32, SPC, 32], F32)

        def conv_phase(s):
            x_t = xpool.tile([128, XW], F32R)
            nc.gpsimd.dma_start(x_t[:], xs[s, :, :])
            for t in range(NCH):
                pc = ps.tile([128, 1024], F32, tag="pA", bufs=2)
                for h in range(2):
                    for kpos in range(4):
                        kh, kw = kpos // 2, kpos % 2
                        off = (2 * t + kh) * W_SP + kw
                        nc.tensor.matmul(
                            pc[0:NV, 512 * h:512 * (h + 1)],
                            x_t[:, off:off + NV],
                            wt_t[:, kpos * OUT_C + 512 * h: kpos * OUT_C + 512 * (h + 1)],
                            start=(kpos == 0), stop=(kpos == 3))
                nc.vector.tensor_add(ut[0:NV, t, :], pc[0:NV, :], bias_t[0:NV, :])

        if KREPS > 1:
            rep_ctx = tc.For_i(0, KREPS, 1)
            rep_ctx.__enter__()

        conv_phase(0)
        for s in range(SPC):
            # ---- transpose into u_A; piggyback row-sums for o0 ----
            o0p = work.tile([128, 8, 4], F32, tag="o0p")
            for g in range(8):
                for b4 in range(4):
                    ptr_t = ps.tile([128, 8, 128], BF16, tag="pmix", bufs=2)
                    for j in range(8):
                        t = 8 * b4 + j
                        nc.tensor.transpose(ptr_t[:, j, 0:NV],
                                            ut[0:NV, t, 128 * g:128 * (g + 1)],
                                            eye[0:NV, 0:NV])
                    nc.scalar.activation(ua[:, g, 8 * b4:8 * b4 + 8, 0:NV],
                                         ptr_t[:, :, 0:NV], AF.Copy,
                                         accum_out=o0p[:, g, b4:b4 + 1])

            # ---- o0 (uniform softmax iteration) ----
            o0ch = work.tile([128, 8], F32, tag="o0ch")
            nc.vector.tensor_reduce(o0ch[:], o0p[:], axis=mybir.AxisListType.X,
                                    op=mybir.AluOpType.add)
            # subtract fake-position contributions (col 63 of every chunk; cols
            # 64:127 of the last chunk)
            f1 = work.tile([128, 8], F32, tag="f1")
            nc.vector.tensor_reduce(f1[:], ua[:, :, :, 63],
                                    axis=mybir.AxisListType.X, op=mybir.AluOpType.add)
            f2 = work.tile([128, 8], F32, tag="f2")
            nc.vector.tensor_reduce(f2[:], ua[:, :, NCH - 1, 64:NV],
                                    axis=mybir.AxisListType.X, op=mybir.AluOpType.add)
            nc.vector.tensor_sub(o0ch[:], o0ch[:], f1[:])
            nc.vector.tensor_sub(o0ch[:], o0ch[:], f2[:])
            # reshuffle row-sums [oc%128, g] -> [i, jk]
            lhsT5 = work.tile([128, 32], BF16, tag="lhsT5")
            nc.vector.tensor_tensor(
                lhsT5[:].rearrange("p (g q) -> p g q", q=4),
                o0ch[:].unsqueeze(2).broadcast_to([128, 8, 4]),
                mask2.rearrange("p (g q) -> p g q", q=4),
                op=mybir.AluOpType.mult)
            o_ps = ps.tile([32, 32], F32, tag="pA", bufs=2)
            nc.tensor.matmul(o_ps[:], lhsT5[:], rhs5, start=True, stop=True)
            o0_sb = work.tile([32, 32], F32, tag="onx")
            nc.vector.tensor_copy(o0_sb[:], o_ps[:])
            o_cur = o0_sb  # [32 i, 32 jk] f32

            # ---- routing iterations ----
            for it in range(2):
                # normalize o -> o' (bf16); rsqrt via ln/exp (single ACT table set)
                ssq = work.tile([32, 1], F32, tag="s1")
                tmp32 = work.tile([32, 32], F32, tag="t32")
                nc.vector.tensor_tensor(tmp32[:], o_cur[:], o_cur[:],
                                        op=mybir.AluOpType.mult)
                nc.vector.tensor_reduce(ssq[:], tmp32[:], axis=mybir.AxisListType.X,
                                        op=mybir.AluOpType.add)
                lns = work.tile([32, 1], F32, tag="s2")
                nc.scalar.activation(lns[:], ssq[:], AF.Ln)
                rn = work.tile([32, 1], F32, tag="s3")
                nc.scalar.activation(rn[:], lns[:], AF.Exp, scale=-0.5)
                o_n = work.tile([32, 32], BF16, tag="on")
                nc.scalar.activation(o_n[:], o_cur[:], AF.Copy, scale=rn[:])

                # ObdT: transpose o', strip-replicate via matmul, mask per group
                oT_ps = ps.tile([32, 32], BF16, tag="pmix", bufs=2)
                nc.tensor.transpose(oT_ps[:], o_n[:], eye[0:32, 0:32])
                oT_sb = work.tile([32, 32], BF16, tag="oT")
                nc.vector.tensor_copy(oT_sb[:], oT_ps[:])
                s_ps = ps.tile([128, 32], F32, tag="pA", bufs=2)
                nc.tensor.matmul(s_ps[:], irep, oT_sb[:], start=True, stop=True)
                obdt = work.tile([128, 8, 32], BF16, tag="obdt")
                for g in range(8):
                    nc.vector.tensor_tensor(
                        obdt[:, g, :], s_ps[:],
                        maskbd.rearrange("p (g i) -> p g i", i=32)[:, g, :],
                        op=mybir.AluOpType.mult)

                # fused pass over NG groups of GC chunks: b-mm -> softmax -> o-mm
                po = ps.tile([32, 1024], F32, tag="po", bufs=1)
                cts = [None] * NG

                def o_mms(grp):
                    for j in range(GC):
                        t = GC * grp + j
                        for h in range(2):
                            nc.tensor.matmul(
                                po[:, 512 * h:512 * (h + 1)],
                                cts[grp][0:NV, j, :],
                                ut[0:NV, t, 512 * h:512 * (h + 1)],
                                start=(t == 0), stop=(t == NCH - 1))

                for grp in range(NG):
                    pb_t = ps.tile([128, GC, 32], F32, tag="pmix", bufs=2)
                    for j in range(GC):
                        t = GC * grp + j
                        for g in range(8):
                            nc.tensor.matmul(pb_t[:, j, :], ua[:, g, t, :],
                                             obdt[:, g, :],
                                             start=(g == 0), stop=(g == 7))
                    e_g = work.tile([128, GC, 32], F32, tag="eg")
                    nc.scalar.activation(e_g[0:NV], pb_t[0:NV], AF.Exp)
                    z_g = work.tile([128, GC], F32, tag="zg")
                    nc.vector.tensor_reduce(z_g[0:NV], e_g[0:NV],
                                            axis=mybir.AxisListType.X,
                                            op=mybir.AluOpType.add)
                    zi_g = work.tile([128, GC], F32, tag="zig")
                    nc.vector.reciprocal(zi_g[0:NV], z_g[0:NV])
                    # zero the fake rows' routing weights: row 63 of every
                    # chunk; rows 64:127 of the last chunk (rmB, last col)
                    zi2 = work.tile([128, GC], F32, tag="zi2")
                    nc.vector.tensor_tensor(zi2[0:NV], zi_g[0:NV],
                                            (rmB if grp == NG - 1 else rmA)[0:NV, :],
                                            op=mybir.AluOpType.mult)
                    ct = work.tile([128, GC, 32], BF16, tag="ct")
                    nc.vector.tensor_tensor(
                        ct[0:NV], e_g[0:NV],
                        zi2[0:NV].unsqueeze(2).broadcast_to([NV, GC, 32]),
                        op=mybir.AluOpType.mult)
                    cts[grp] = ct
                    if grp > 0:
                        o_mms(grp - 1)
                o_mms(NG - 1)

                if it == 1 and s + 1 < SPC:
                    conv_phase(s + 1)  # overlap next conv with this tail

                # diagonal extraction
                tmpd = work.tile([32, 1024], F32, tag="tmpd")
                nc.vector.tensor_tensor(tmpd[:], po[:], maskdiag,
                                        op=mybir.AluOpType.mult)
                o_nx = work.tile([32, 32], F32, tag="onx")
                nc.vector.tensor_reduce(o_nx[:],
                                        tmpd[:].rearrange("p (i k) -> p k i", k=32),
                                        axis=mybir.AxisListType.X,
                                        op=mybir.AluOpType.add)
                o_cur = o_nx

            # ---- squash ----
            ssq = work.tile([32, 1], F32, tag="s1")
            tmp32 = work.tile([32, 32], F32, tag="t32")
            nc.vector.tensor_tensor(tmp32[:], o_cur[:], o_cur[:],
                                    op=mybir.AluOpType.mult)
            nc.vector.tensor_reduce(ssq[:], tmp32[:], axis=mybir.AxisListType.X,
                                    op=mybir.AluOpType.add)
            lns = work.tile([32, 1], F32, tag="s2")
            nc.scalar.activation(lns[:], ssq[:], AF.Ln)
            sq_s = work.tile([32, 1], F32, tag="s3")
            nc.scalar.activation(sq_s[:], lns[:], AF.Exp, scale=0.5)
            d2 = work.tile([32, 1], F32, tag="s4")
            nc.vector.tensor_scalar_add(d2[:], sq_s[:], 1e-6)
            r2 = work.tile([32, 1], F32, tag="s5")
            nc.vector.reciprocal(r2[:], d2[:])
            p1 = work.tile([32, 1], F32, tag="s6")
            nc.vector.tensor_scalar_add(p1[:], ssq[:], 1.0)
            r1 = work.tile([32, 1], F32, tag="s7")
            nc.vector.reciprocal(r1[:], p1[:])
            t1 = work.tile([32, 1], F32, tag="s8")
            nc.vector.tensor_tensor(t1[:], ssq[:], r1[:], op=mybir.AluOpType.mult)
            f = work.tile([32, 1], F32, tag="s9")
            nc.vector.tensor_tensor(f[:], t1[:], r2[:], op=mybir.AluOpType.mult)
            nc.scalar.activation(out_sb[:, s, :], o_cur[:], AF.Copy, scale=f[:])

        if KREPS > 1:
            rep_ctx.__exit__(None, None, None)

        nc.gpsimd.dma_start(out_d.rearrange("s i j -> i s j"), out_sb[:])


def _consts():
    p = np.arange(128)
    i = np.arange(32)
    g = np.arange(8)
    maskbd = (i[None, None, :] == 4 * g[None, :, None] + p[:, None, None] // 32)
    mask2 = (p[:, None] // 32 == i[None, :] % 4)
    ch = np.arange(OUT_C)
    maskdiag = (ch[None, :] // 32 == i[:, None])
    maskdiag_p = np.zeros((128, OUT_C), np.float32)
    maskdiag_p[0:32] = maskdiag
    rmA = np.ones((128, GC), np.float32)
    rmA[63] = 0.0
    rmB = rmA.copy()
    rmB[64:, GC - 1] = 0.0
    cons_f = np.concatenate(
        [maskbd.reshape(128, 256).astype(np.float32),
         mask2.astype(np.float32), maskdiag_p, rmA, rmB], axis=1)

    eye = np.eye(128, dtype=np.float32)
    irep_p = np.zeros((128, 128), np.float32)
    irep_p[0:32] = (np.arange(32)[:, None] == p[None, :] % 32)   # [q, p]
    rhs5 = (p[:, None] % 32 == i[None, :]).astype(np.float32)     # [p, jk]
    cons_h = np.concatenate([eye, irep_p, rhs5], axis=1).astype(ml_dtypes.bfloat16)
    return cons_f.astype(np.float32), cons_h


def kernel(x, W, b_conv):
    from concourse.bass_utils import run_bass_kernel_spmd

    x = np.asarray(x, dtype=np.float32)
    W = np.asarray(W, dtype=np.float32)
    b_conv = np.asarray(b_conv, dtype=np.float32)

    # Wt[c, kpos*1024 + oc] = W[oc, c, kh, kw]
    wt = np.ascontiguousarray(
        W.reshape(OUT_C, IN_C, 4).transpose(1, 2, 0).reshape(IN_C, 4 * OUT_C))
    bias_bc = np.broadcast_to(b_conv, (128, OUT_C)).copy()
    cons_f, cons_h = _consts()

    if "nc" not in _BUILT:
        _BUILT["nc"] = _build_nc()
    nc = _BUILT["nc"]

    xp = np.zeros((B, 128, XW), np.float32)
    xp[:, :, :H * W_SP] = x.reshape(B, 128, H * W_SP)

    in_maps = []
    for c in range(N_CORES):
        in_maps.append({"xs": np.ascontiguousarray(xp[c * SPC:(c + 1) * SPC]),
                        "wt": wt, "bias_bc": bias_bc,
                        "cons_f": cons_f, "cons_h": cons_h})

    global _last_in_maps
    _last_in_maps = in_maps
    res = run_bass_kernel_spmd(nc, in_maps, core_ids=list(range(N_CORES)))
    out = np.concatenate([r["out"] for r in res.results], axis=0)
    return out.astype(np.float32)


_last_in_maps = None

